# revision 1
# baseline (speedup 1.0000x reference)
"""Bernstein flow density kernel for 8x TRN2 NeuronCores.

Math (per sample n):
  density(n) = prod_i [ phi_i[n,15] + sum_m tf_i[n,m] * psi_i[n,m] ]
  tf_i = cond_i @ c_alpha_i,  cond_i = B_0 (x) ... (x) B_{i-1}  (row-wise Kron)
Key identity: Bernstein bases sum to 1, so cond_i is a marginal of
cond_5 [N,1024]; all six matmuls merge into ONE:
  tf_all[N, 90] = cond_5 @ W,  W[c, i*15+m] = c_alpha_i[c >> 2*(5-i), m]
psi_i[n,m] = phi_i[n,m] - phi_i[n,m+1] (m=0..14), phi = scaled Bernstein deg-15.

Per core (8192 samples, p-major: local n = p*64 + s):
  1. build deg-3 factor tables B_j [128,(s,j,a)] with vector ops
  2. per s-tile: cond_5 [128,1024] via 4 broadcast-AP tensor_tensor ops
  3. PE-transpose 128x128 blocks -> cond^T chunks; fp32 matmul vs W -> tf^T
  4. PE-transpose tf^T back to natural; build phi/psi; combine + 6-way product
"""

import math
import sys

import numpy as np

sys.path.insert(0, "/opt/trn_rl_repo")

import concourse.bacc as bacc  # noqa: E402
import concourse.bass as bass  # noqa: E402
import concourse.tile as tile  # noqa: E402
from concourse import mybir  # noqa: E402
from concourse.bass_utils import run_bass_kernel_spmd  # noqa: E402

N = 65536
DIM = 6
NCORES = 8
NC = N // NCORES          # 8192 samples per core
P = 128
S = NC // P               # 64 samples per partition
NT = 4                    # s-tiles per matmul group
NG = S // NT              # 16 groups (matmul chunks of 512 samples)
NB = NT * P               # 512 samples per group
CDIM = 1024               # cond_5 width
KCH = CDIM // P           # 8 contraction chunks
M90 = 90                  # 6 dims * 15 coeffs

F32 = mybir.dt.float32
F32R = mybir.dt.float32r
MUL = mybir.AluOpType.mult
ADD = mybir.AluOpType.add
SUB = mybir.AluOpType.subtract

_CACHE = {}


def _ap(a, off_elems, dims):
    """AP over slice a with replaced free dims; dims = [[step,count],...]."""
    return bass.AP(tensor=a.tensor, offset=a.offset + off_elems, ap=[a.ap[0]] + dims)


def _build_nc(mm_dtype=F32):
    nc = bacc.Bacc(target_bir_lowering=False, trn_type="TRN2")

    xr = nc.dram_tensor("xr", [P, S, DIM], F32, kind="ExternalInput")
    wmat = nc.dram_tensor("wmat", [CDIM, M90], F32, kind="ExternalInput")
    kap = nc.dram_tensor("kap", [1, 16], F32, kind="ExternalInput")
    ident = nc.dram_tensor("ident", [P, P], F32, kind="ExternalInput")
    dens_out = nc.dram_tensor("dens", [P, S], F32, kind="ExternalOutput")

    with tile.TileContext(nc) as tc:
        with (
            tc.tile_pool(name="singles", bufs=1) as singles,
            tc.tile_pool(name="bigs", bufs=1) as bigs,
            tc.tile_pool(name="cond", bufs=4) as condp,
            tc.tile_pool(name="ctb", bufs=3) as ctbp,
            tc.tile_pool(name="pows", bufs=3) as powp,
            tc.tile_pool(name="ps_t", bufs=2, space="PSUM") as ps_t,
            tc.tile_pool(name="ps_mm", bufs=2, space="PSUM") as ps_mm,
            tc.tile_pool(name="ps_x", bufs=1, space="PSUM") as ps_x,
        ):
            # ---- constants / inputs ----
            xin = singles.tile([P, S, DIM], F32)
            nc.sync.dma_start(out=xin[:, :, :], in_=xr[:, :, :])
            wsb = singles.tile([P, KCH, M90], F32)
            nc.sync.dma_start(
                out=wsb[:, :, :],
                in_=bass.AP(tensor=wmat[:, :].tensor, offset=0,
                            ap=[[M90, P], [P * M90, KCH], [1, M90]]),
            )
            idn = singles.tile([P, P], F32)
            nc.sync.dma_start(out=idn[:, :], in_=ident[:, :])
            kapt = singles.tile([P, 16], F32)
            nc.sync.dma_start(
                out=kapt[:, :],
                in_=bass.AP(tensor=kap[:, :].tensor, offset=0, ap=[[0, P], [1, 16]]),
            )

            # PE "pre-observe" dummies: walrus fp32 fused matmul (LDW+MM)
            # tolerates only one sync wait, so make the PE observe the DMA
            # semaphores up front via tiny throwaway transposes.
            scr = ps_x.tile([2, 2], F32)
            nc.tensor.matmul(out=scr[:, :], lhsT=idn[:2, :2], rhs=idn[:2, :2],
                             is_transpose=True, start=True, stop=True,
                             skip_group_check=True)
            nc.tensor.matmul(out=scr[:, :], lhsT=wsb[:2, 0, :2], rhs=idn[:2, :2],
                             is_transpose=True, start=True, stop=True,
                             skip_group_check=True)

            xa = xin[:, :, :]

            # ---- stage A: powers of x, 1-x ----
            FD6 = S * DIM
            omx = singles.tile([P, S, DIM], F32)
            x2 = singles.tile([P, S, DIM], F32)
            x3 = singles.tile([P, S, DIM], F32)
            omx2 = singles.tile([P, S, DIM], F32)
            omx3 = singles.tile([P, S, DIM], F32)
            # omx = (x * -1) + 1
            nc.vector.tensor_scalar(
                out=omx[:, :, :], in0=xa, scalar1=-1.0, scalar2=1.0, op0=MUL, op1=ADD
            )
            nc.vector.tensor_tensor(out=x2[:, :, :], in0=xa, in1=xa, op=MUL)
            nc.vector.tensor_tensor(
                out=omx2[:, :, :], in0=omx[:, :, :], in1=omx[:, :, :], op=MUL
            )
            nc.vector.tensor_tensor(out=x3[:, :, :], in0=x2[:, :, :], in1=xa, op=MUL)
            nc.vector.tensor_tensor(
                out=omx3[:, :, :], in0=omx2[:, :, :], in1=omx[:, :, :], op=MUL
            )

            # ---- stage B: deg-3 tables Bbig[p, s, j, a]  j=0..4 ----
            NJ = 5
            Bbig = singles.tile([P, S, NJ, 4], F32)
            for (a, src, scl, other) in (
                (0, omx3, None, None),
                (1, xin, 3.0, omx2),
                (2, x2, 3.0, omx),
                (3, x3, None, None),
            ):
                src_ap = _ap(src[:, :, :], 0, [[DIM, S], [1, NJ]])
                out_ap = _ap(Bbig[:, :, :, :], a, [[4 * NJ, S], [4, NJ]])
                if scl is None:
                    nc.vector.tensor_copy(out=out_ap, in_=src_ap)
                else:
                    nc.vector.scalar_tensor_tensor(
                        out=out_ap, in0=src_ap, scalar=scl,
                        in1=_ap(other[:, :, :], 0, [[DIM, S], [1, NJ]]),
                        op0=MUL, op1=MUL,
                    )

            # ---- stage C+D: cond tiles, transpose, matmul per group ----
            tf_big = bigs.tile([P, S, M90], F32)   # natural-layout tf
            psi_big = bigs.tile([P, S, DIM, 15], F32)
            p15 = bigs.tile([P, S, DIM], F32)
            ebig = bigs.tile([P, S, DIM, 15], F32)
            sred = bigs.tile([P, S, DIM], F32)

            def emit_phipsi(j):
                px = powp.tile([P, 16, S], F32, tag="px")
                pq = powp.tile([P, 16, S], F32, tag="pq")
                for (tbl, base) in ((px, xin), (pq, omx)):
                    nc.vector.memset(tbl[:, 0, :], 1.0)
                    nc.vector.tensor_copy(
                        out=tbl[:, 1, :], in_=_ap(base[:, :, :], j, [[DIM, S]]))
                    t1 = tbl[:, :, :]
                    nc.vector.tensor_tensor(
                        out=tbl[:, 2, :], in0=t1[:, 1, :], in1=t1[:, 1, :], op=MUL)
                    nc.vector.tensor_tensor(
                        out=_ap(t1, 3 * S, [[1, 2 * S]]),
                        in0=_ap(t1, S, [[1, 2 * S]]),
                        in1=_ap(t1, 2 * S, [[0, 2], [1, S]]), op=MUL)
                    nc.any.tensor_tensor(
                        out=_ap(t1, 5 * S, [[1, 4 * S]]),
                        in0=_ap(t1, S, [[1, 4 * S]]),
                        in1=_ap(t1, 4 * S, [[0, 4], [1, S]]), op=MUL)
                    nc.any.tensor_tensor(
                        out=_ap(t1, 9 * S, [[1, 7 * S]]),
                        in0=_ap(t1, S, [[1, 7 * S]]),
                        in1=_ap(t1, 8 * S, [[0, 7], [1, S]]), op=MUL)
                v = powp.tile([P, 16, S], F32, tag="v")
                nc.any.tensor_tensor(
                    out=v[:, :, :],
                    in0=px[:, :, :],
                    in1=_ap(pq[:, :, :], 15 * S, [[-S, 16], [1, S]]), op=MUL)
                nc.any.tensor_tensor(
                    out=v[:, :, :], in0=v[:, :, :],
                    in1=_ap(kapt[:, :], 0, [[1, 16], [0, S]]), op=MUL)
                nc.any.tensor_tensor(
                    out=_ap(psi_big[:, :, :, :], j * 15, [[DIM * 15, S], [1, 15]]),
                    in0=_ap(v[:, :, :], 0, [[1, S], [S, 15]]),
                    in1=_ap(v[:, :, :], S, [[1, S], [S, 15]]), op=SUB)
                nc.vector.tensor_copy(
                    out=_ap(p15[:, :, :], j, [[DIM, S]]),
                    in_=_ap(v[:, :, :], 15 * S, [[1, S]]))

            def emit_combine(s0, s1):
                ns = s1 - s0
                nc.any.tensor_tensor(
                    out=_ap(ebig[:, :, :, :], s0 * M90, [[1, ns * M90]]),
                    in0=_ap(tf_big[:, :, :], s0 * M90, [[1, ns * M90]]),
                    in1=_ap(psi_big[:, :, :, :], s0 * M90, [[1, ns * M90]]), op=MUL)
                nc.vector.tensor_reduce(
                    out=_ap(sred[:, :, :], s0 * DIM, [[1, ns * DIM]]),
                    in_=_ap(ebig[:, :, :, :], s0 * M90, [[15, ns * DIM], [1, 15]]),
                    op=ADD, axis=mybir.AxisListType.X)

            for g in range(NG):
                ctb = ctbp.tile([P, KCH, NT, P], F32, tag="ctb")
                bb = Bbig[:, :, :, :]
                gb = g * NT * NJ * 4   # B-table offset of this group's tiles
                TS = NJ * 4            # per-tile stride in Bbig cols
                k2g = condp.tile([P, NT, 16], F32, tag="k2")
                k3g = condp.tile([P, NT, 64], F32, tag="k3")
                q34g = condp.tile([P, NT, 16], F32, tag="q34")
                nc.vector.tensor_tensor(
                    out=k2g[:, :, :],
                    in0=_ap(bb, gb + 0, [[TS, NT], [1, 4], [0, 4]]),
                    in1=_ap(bb, gb + 4, [[TS, NT], [0, 4], [1, 4]]), op=MUL)
                nc.vector.tensor_tensor(
                    out=k3g[:, :, :],
                    in0=_ap(k2g[:, :, :], 0, [[16, NT], [1, 16], [0, 4]]),
                    in1=_ap(bb, gb + 8, [[TS, NT], [0, 16], [1, 4]]), op=MUL)
                nc.vector.tensor_tensor(
                    out=q34g[:, :, :],
                    in0=_ap(bb, gb + 12, [[TS, NT], [1, 4], [0, 4]]),
                    in1=_ap(bb, gb + 16, [[TS, NT], [0, 4], [1, 4]]), op=MUL)
                for t in range(NT):
                    cnd = condp.tile([P, CDIM], F32, tag="cond")
                    nc.any.tensor_tensor(
                        out=cnd[:, :],
                        in0=_ap(k3g[:, :, :], t * 64, [[1, 64], [0, 16]]),
                        in1=_ap(q34g[:, :, :], t * 16, [[0, 64], [1, 16]]), op=MUL)

                    # transpose 8 128x128 blocks -> 2-bank psum tile
                    pst = ps_t.tile([P, KCH, P], F32, tag="pst")
                    # dummy absorbs the psum-slot-release wait so the first
                    # real transpose carries only the DVE (cond) wait
                    nc.tensor.matmul(out=pst[:2, 0, :2], lhsT=idn[:2, :2],
                                     rhs=idn[:2, :2], is_transpose=True,
                                     start=True, stop=True,
                                     skip_group_check=True)
                    for k in range(KCH):
                        nc.tensor.matmul(
                            out=pst[:, k, :],
                            lhsT=cnd[:, k * P:(k + 1) * P],
                            rhs=idn[:, :],
                            is_transpose=True,
                            start=(k % 4 == 0),
                            stop=(k % 4 == 3),
                        )
                    # copy psum -> condT sbuf (scatter over k, col t*128)
                    for half in range(2):
                        nc.scalar.copy(
                            out=_ap(ctb[:, :, :, :], (half * 4) * NT * P + t * P,
                                    [[NT * P, 4], [1, P]]),
                            in_=_ap(pst[:, :, :], half * 4 * P, [[P, 4], [1, P]]),
                        )

                # tf natural directly: stationary cond^T, moving W (90 cols)
                for t in range(NT):
                    pmm = ps_mm.tile([P, M90], F32, tag="pmm")
                    for k in range(KCH):
                        nc.tensor.matmul(
                            out=pmm[:, :],
                            lhsT=ctb[:, k, t, :],
                            rhs=wsb[:, k, :],
                            start=(k == 0),
                            stop=(k == KCH - 1),
                        )
                    nc.scalar.copy(
                        out=_ap(tf_big[:, :, :], (g * NT + t) * M90, [[1, M90]]),
                        in_=pmm[:, :],
                    )
                pp = {0: 0, 1: 1, 3: 2, 5: 3, 7: 4, 8: 5}
                if g in pp:
                    emit_phipsi(pp[g])
                elif g in (9, 11, 13, 15):
                    # psi complete after group 8; combine finished s-ranges
                    done = {9: (0, 24), 11: (24, 40), 13: (40, 56),
                            15: (56, 64)}[g]
                    emit_combine(*done)

            # ---- stage F: final combine tail ----
            nc.vector.tensor_tensor(
                out=sred[:, :, :], in0=sred[:, :, :], in1=p15[:, :, :], op=ADD)
            t1 = bigs.tile([P, S, 3], F32)
            nc.vector.tensor_tensor(
                out=t1[:, :, :],
                in0=_ap(sred[:, :, :], 0, [[DIM, S], [2, 3]]),
                in1=_ap(sred[:, :, :], 1, [[DIM, S], [2, 3]]), op=MUL)
            dq = bigs.tile([P, S], F32)
            nc.vector.tensor_tensor(
                out=dq[:, :],
                in0=_ap(t1[:, :, :], 0, [[3, S]]),
                in1=_ap(t1[:, :, :], 1, [[3, S]]), op=MUL)
            nc.vector.tensor_tensor(
                out=dq[:, :], in0=dq[:, :],
                in1=_ap(t1[:, :, :], 2, [[3, S]]), op=MUL)
            nc.sync.dma_start(out=dens_out[:, :], in_=dq[:, :])

    nc.finalize()
    return nc


def _softplus64(v):
    return np.logaddexp(0.0, v)


def _host_w(As):
    cols = []
    for i in range(DIM):
        c = np.cumsum(_softplus64(As[i].astype(np.float64)), axis=1)
        ca = 2.0 * (1.0 / (1.0 + np.exp(-c)) - 0.5)
        cols.append(np.repeat(ca, 4 ** (5 - i), axis=0))
    return np.concatenate(cols, axis=1).astype(np.float32)


def kernel(**inputs):
    x = np.asarray(inputs["x"], dtype=np.float32)
    As = [np.asarray(inputs[f"A{i}"], dtype=np.float32) for i in range(DIM)]

    if "nc" not in _CACHE:
        _CACHE["nc"] = _build_nc()
    nc = _CACHE["nc"]

    w = _host_w(As)
    kapv = (16.0 * np.array([math.comb(15, a) for a in range(16)],
                            dtype=np.float64)).astype(np.float32)[None, :]
    idn = np.eye(P, dtype=np.float32)

    in_maps = []
    for c in range(NCORES):
        xc = x[c * NC:(c + 1) * NC].reshape(P, S, DIM)
        in_maps.append({"xr": xc, "wmat": w, "kap": kapv, "ident": idn})

    res = run_bass_kernel_spmd(nc, in_maps, core_ids=list(range(NCORES)))
    outs = [r["dens"].reshape(NC) for r in res.results]
    return np.concatenate(outs, axis=0)


if __name__ == "__main__":
    rng = np.random.default_rng(0)
    ins = {"x": rng.uniform(0, 1, (N, DIM)).astype(np.float32)}
    for i in range(DIM):
        ins[f"A{i}"] = rng.uniform(0, 1, ((4 ** i), 15)).astype(np.float32)
    out = kernel(**ins)
    print(out.shape, out[:4])



# revision 2
# speedup vs baseline: 1.0986x; 1.0986x over previous
"""Bernstein flow density kernel v2 for 8x TRN2 NeuronCores.

Math (per sample n):  density(n) = prod_i s_i,
  s_i = sum_{m=0..15} W''_i-contraction:  s_i = sum_m dk_i[n,m] * x_i^m * (1-x_i)^(15-m)
  dk_i[n,:] = cond_i[n,:] @ W''_i,  W''_i[r,m] = kap[m]*(ca_i[r,m]-ca_i[r,m-1])
  (ca[,-1]=0, ca[,15]=1; folds the psi-difference + binomial scaling into W'').

Per core (8192 samples, p-major: n = p*64 + s), all fp16 on-chip except x/psum:
  - DVE: deg-3 tables, Kron chain k2->k3->cond4 (s-innermost, 2x fp16),
    G5 combine (B4-weighted l-sum), combine mults + tree reductions.
  - PE: batched fp16 transposes of cond4 chunks; per-slot matmuls:
    dims0-4 fold into one cond4^T contraction (partition-of-unity marginals
    baked into wcomb rows); dim5 via G5 = cond4^T @ ca5w.
  - Act: PSUM->SBUF drains (fp16 casts).
  - Pool: x^m / (1-x)^m power tables (phi-hat factors).
"""

import math
import sys

import numpy as np

sys.path.insert(0, "/opt/trn_rl_repo")

import concourse.bacc as bacc  # noqa: E402
import concourse.bass as bass  # noqa: E402
import concourse.tile as tile  # noqa: E402
from concourse import mybir  # noqa: E402
from concourse.bass_utils import run_bass_kernel_spmd  # noqa: E402

N = 65536
DIM = 6
NCORES = 8
NC = N // NCORES          # 8192 samples per core
P = 128
S = NC // P               # 64 samples per partition
G = 8                     # slots per group
NG = S // G               # 8 groups

F32 = mybir.dt.float32
F16 = mybir.dt.float16
MUL = mybir.AluOpType.mult
ADD = mybir.AluOpType.add

_CACHE = {}

# per-group engine for the ct4 drain: 'a'=Act, 'v'=DVE, 'p'=Pool
CT4_DRAIN = "aaaaaaaa"
# per-group engine for the pdk drain
PDK_DRAIN = "aaaaaaaa"
# per-group engine for the cond4 build ('v' or 'p')
C4_ENG = "vvppppvv"
# per-group engine for the t5 multiply
T5_ENG = "vvvvvvvv"
FAST_START = True
PTR_BUFS = 2
PDK_BUFS = 1
# g -> list of (slot_begin, slot_end) for e1/e2 combine and trees+product
COMBINE_SCHED = {1: [(0, 16)], 3: [(16, 32)], 5: [(32, 48)], 7: [(48, 64)]}
TREE_SCHED = {3: [(0, 32)], 5: [(32, 48)], 7: [(48, 64)]}
C4_EARLY = True     # build c4 for late DVE-groups during the early stall
CG_BUFS = 4
DENS_SPLIT = True
G0_SPLIT = True


def _ap(a, off_elems, dims):
    return bass.AP(tensor=a.tensor, offset=a.offset + off_elems, ap=[a.ap[0]] + dims)


def _build_nc():
    nc = bacc.Bacc(target_bir_lowering=False, trn_type="TRN2")

    xr = nc.dram_tensor("xr", [P, S, DIM], F32, kind="ExternalInput")
    wcomb_d = nc.dram_tensor("wcomb", [P, 2, 80], F16, kind="ExternalInput")
    ca5w_d = nc.dram_tensor("ca5w", [P, 2, 64], F16, kind="ExternalInput")
    idh_d = nc.dram_tensor("idh", [P, P], F16, kind="ExternalInput")
    dens_out = nc.dram_tensor("dens", [P, S], F32, kind="ExternalOutput")

    with tile.TileContext(nc) as tc:
        with (
            tc.tile_pool(name="singles", bufs=1) as sg,
            tc.tile_pool(name="cgp", bufs=CG_BUFS) as cgp,
            tc.tile_pool(name="ct4p", bufs=2) as ct4p,
            tc.tile_pool(name="scr", bufs=2) as scr,
            tc.tile_pool(name="ptr", bufs=PTR_BUFS, space="PSUM") as ptrp,
            tc.tile_pool(name="pg5", bufs=2, space="PSUM") as pg5p,
            tc.tile_pool(name="pdk", bufs=PDK_BUFS, space="PSUM") as pdkp,
        ):
            # ---- inputs / consts ----
            xin = sg.tile([P, S, DIM], F32)
            nc.sync.dma_start(out=xin[:, :, :], in_=xr[:, :, :])
            wcomb = sg.tile([P, 2, 80], F16)
            nc.sync.dma_start(out=wcomb[:, :, :], in_=wcomb_d[:, :, :])
            ca5w = sg.tile([P, 2, 64], F16)
            nc.sync.dma_start(out=ca5w[:, :, :], in_=ca5w_d[:, :, :])
            idh = sg.tile([P, P], F16)
            nc.sync.dma_start(out=idh[:, :], in_=idh_d[:, :])

            # ---- casts ----
            # (s,d)-layout fp16 x and 1-x for power chains (on Act: idle early)
            xh6 = sg.tile([P, S, DIM], F16)
            nc.scalar.copy(out=xh6[:, :, :], in_=xin[:, :, :])
            qh6 = sg.tile([P, S, DIM], F16)
            nc.scalar.activation(out=qh6[:, :, :], in_=xin[:, :, :],
                                 func=mybir.ActivationFunctionType.Copy,
                                 scale=-1.0, bias=1.0)
            # d-major fp16 x / 1-x for deg-3 tables (j = 0..4 only)
            NJ = 5
            xh5 = sg.tile([P, NJ, S], F16)
            qh5 = sg.tile([P, NJ, S], F16)
            x2h = sg.tile([P, NJ, S], F16)
            x3h = sg.tile([P, NJ, S], F16)
            q2h = sg.tile([P, NJ, S], F16)
            q3h = sg.tile([P, NJ, S], F16)
            Bbig = sg.tile([P, NJ, 4, S], F16)

            def emit_prep(sl, w):
                """casts + deg-3 Bernstein tables for slot range [sl, sl+w)."""
                xj = _ap(xin[:, :, :], sl * DIM, [[1, NJ], [DIM, w]])
                d5 = [[S, NJ], [1, w]]
                db = [[4 * S, NJ], [1, w]]
                xs = _ap(xh5[:, :, :], sl, d5)
                qs = _ap(qh5[:, :, :], sl, d5)
                x2s = _ap(x2h[:, :, :], sl, d5)
                q2s = _ap(q2h[:, :, :], sl, d5)
                nc.vector.tensor_copy(out=xs, in_=xj)
                nc.vector.tensor_scalar(out=qs, in0=xj, scalar1=-1.0,
                                        scalar2=1.0, op0=MUL, op1=ADD)
                nc.vector.tensor_tensor(out=x2s, in0=xs, in1=xs, op=MUL)
                nc.vector.tensor_tensor(out=q2s, in0=qs, in1=qs, op=MUL)
                # cubic terms written straight into Bbig (a=3: x^3, a=0: q^3)
                nc.vector.tensor_tensor(
                    out=_ap(Bbig[:, :, :, :], 3 * S + sl, db),
                    in0=x2s, in1=xs, op=MUL)
                nc.vector.tensor_tensor(
                    out=_ap(Bbig[:, :, :, :], 0 * S + sl, db),
                    in0=q2s, in1=qs, op=MUL)
                nc.vector.scalar_tensor_tensor(
                    out=_ap(Bbig[:, :, :, :], 1 * S + sl, db),
                    in0=xs, scalar=3.0, in1=q2s, op0=MUL, op1=MUL)
                nc.vector.scalar_tensor_tensor(
                    out=_ap(Bbig[:, :, :, :], 2 * S + sl, db),
                    in0=x2s, scalar=3.0, in1=qs, op0=MUL, op1=MUL)

            # ---- power chains on Pool: px/pq [p, s, d, m16] ----
            px = sg.tile([P, S, DIM, 16], F16)
            pq = sg.tile([P, S, DIM, 16], F16)
            SD = S * DIM

            def emit_chain(tbl, lvl1):
                t = tbl[:, :, :, :]
                nc.gpsimd.memset(_ap(t, 0, [[16, SD]]), 1.0)
                nc.gpsimd.tensor_copy(out=_ap(t, 1, [[16, SD]]),
                                      in_=lvl1[:, :, :])
                nc.gpsimd.tensor_tensor(
                    out=_ap(t, 2, [[16, SD]]), in0=_ap(t, 1, [[16, SD]]),
                    in1=_ap(t, 1, [[16, SD]]), op=MUL)
                nc.gpsimd.tensor_tensor(
                    out=_ap(t, 3, [[16, SD], [1, 2]]),
                    in0=_ap(t, 1, [[16, SD], [1, 2]]),
                    in1=_ap(t, 2, [[16, SD], [0, 2]]), op=MUL)
                nc.gpsimd.tensor_tensor(
                    out=_ap(t, 5, [[16, SD], [1, 4]]),
                    in0=_ap(t, 1, [[16, SD], [1, 4]]),
                    in1=_ap(t, 4, [[16, SD], [0, 4]]), op=MUL)
                nc.gpsimd.tensor_tensor(
                    out=_ap(t, 9, [[16, SD], [1, 7]]),
                    in0=_ap(t, 1, [[16, SD], [1, 7]]),
                    in1=_ap(t, 8, [[16, SD], [0, 7]]), op=MUL)

            # ---- Kron chain: k2, k3 ----
            k2 = sg.tile([P, 16, S], F16)      # (a0,a1) major, s inner
            k3 = sg.tile([P, 64, S], F16)      # (a0a1a2) major, s inner

            def emit_k2_range(sl, w):
                nc.vector.tensor_tensor(
                    out=_ap(k2[:, :, :], sl, [[4 * S, 4], [S, 4], [1, w]]),
                    in0=_ap(Bbig[:, :, :, :], sl, [[S, 4], [0, 4], [1, w]]),
                    in1=_ap(Bbig[:, :, :, :], 4 * S + sl,
                            [[0, 4], [S, 4], [1, w]]),
                    op=MUL)

            def emit_k3_range(sl, w):
                nc.vector.tensor_tensor(
                    out=_ap(k3[:, :, :], sl, [[4 * S, 16], [S, 4], [1, w]]),
                    in0=_ap(k2[:, :, :], sl, [[S, 16], [0, 4], [1, w]]),
                    in1=_ap(Bbig[:, :, :, :], 2 * 4 * S + sl,
                            [[0, 16], [S, 4], [1, w]]),
                    op=MUL)

            if FAST_START:
                emit_prep(0, G)
                emit_k2_range(0, G)
                emit_k3_range(0, G)
            else:
                emit_prep(0, S)
                emit_k2_range(0, S)
                emit_k3_range(0, 16)

            # ---- per-slot outputs ----
            dks = sg.tile([P, S, 96], F16)     # dk bands: cols 16i+m, i=0..5
            s6 = sg.tile([P, S, DIM], F16)
            e2 = sg.tile([P, S, 96], F16)
            dq = sg.tile([P, S], F32)

            emit_chain(px, xh6)
            emit_chain(pq, qh6)

            cg_tiles = {}

            def emit_c4(g):
                s0 = g * G
                cg = cgp.tile([P, 256, G], F16, tag=f"cg{g % CG_BUFS}")
                eng = nc.vector if C4_ENG[g] == "v" else nc.gpsimd
                eng.tensor_tensor(
                    out=cg[:, :, :],
                    in0=_ap(k3[:, :, :], s0, [[S, 64], [0, 4], [1, G]]),
                    in1=_ap(Bbig[:, :, :, :], 3 * 4 * S + s0,
                            [[0, 64], [S, 4], [1, G]]),
                    op=MUL)
                cg_tiles[g] = cg

            for g in range(NG):
                s0 = g * G
                if C4_EARLY:
                    k3_sched = {1: (16, 16)}
                else:
                    k3_sched = ({1: (16, 16), 2: (32, 16), 3: (48, 16)}
                                if FAST_START else
                                {2: (16, 16), 4: (32, 16), 6: (48, 16)})
                if g in k3_sched:
                    emit_k3_range(*k3_sched[g])
                # cond4 for this group: [p, c256, ds8]
                # first/last groups on DVE (pipeline start + tail), rest Pool
                ptr = ptrp.tile([P, 16, P], F16, tag="ptr")

                def emit_transposes(cg, dl, dh):
                    for ds in range(dl, dh):
                        for h in range(2):
                            nc.tensor.matmul(
                                out=ptr[:, ds * 2 + h, :],
                                lhsT=_ap(cg[:, :, :], h * P * G + ds, [[G, P]]),
                                rhs=idh[:, :],
                                is_transpose=True, start=True, stop=True)

                if g == 0 and FAST_START and G0_SPLIT:
                    # split build+transpose halves for minimal lead-in
                    cg = cgp.tile([P, 256, G], F16, tag="cg0")
                    cg_tiles[0] = cg
                    for hf in range(2):
                        nc.vector.tensor_tensor(
                            out=_ap(cg[:, :, :], hf * 4, [[G, 256], [1, 4]]),
                            in0=_ap(k3[:, :, :], hf * 4,
                                    [[S, 64], [0, 4], [1, 4]]),
                            in1=_ap(Bbig[:, :, :, :], 3 * 4 * S + hf * 4,
                                    [[0, 64], [S, 4], [1, 4]]),
                            op=MUL)
                        emit_transposes(cg, hf * 4, hf * 4 + 4)
                else:
                    if g not in cg_tiles:
                        emit_c4(g)
                    cg = cg_tiles[g]
                    emit_transposes(cg, 0, G)

                # ct4 drain psum -> sbuf, distributed across engines.
                # lowercase = whole drain; split modes use two engines/halves.
                ct4 = ct4p.tile([P, G, 2, P], F16, tag="ct4")
                mode = CT4_DRAIN[g]
                if g == 0 and FAST_START and G0_SPLIT:
                    halves = [("a", 0, 8), ("a", 8, 8)]
                else:
                    halves = {"a": [("a", 0, 16)], "v": [("v", 0, 16)],
                              "p": [("p", 0, 16)],
                              "s": [("a", 0, 8), ("v", 8, 8)],
                              "t": [("a", 0, 8), ("p", 8, 8)],
                              "u": [("v", 0, 8), ("p", 8, 8)]}[mode]
                for engc, off, w in halves:
                    de = {"a": nc.scalar, "v": nc.vector, "p": nc.gpsimd}[engc]
                    oap = _ap(ct4[:, :, :, :], off * P, [[1, w * P]])
                    iap = _ap(ptr[:, :, :], off * P, [[1, w * P]])
                    if de is nc.scalar:
                        de.copy(out=oap, in_=iap)
                    else:
                        de.tensor_copy(out=oap, in_=iap)

                # matmuls per slot: dims0-4 -> pdk, dim5 G5 -> pg5
                pdk = pdkp.tile([P, G, 128], F32, tag="pdk")
                pg5 = pg5p.tile([P, G, 64], F32, tag="pg5")
                for ds in range(G):
                    for h in range(2):
                        nc.tensor.matmul(
                            out=_ap(pdk[:, :, :], ds * 128, [[1, 80]]),
                            lhsT=ct4[:, ds, h, :], rhs=wcomb[:, h, :],
                            start=(h == 0), stop=(h == 1))
                    for h in range(2):
                        nc.tensor.matmul(
                            out=pg5[:, ds, :],
                            lhsT=ct4[:, ds, h, :], rhs=ca5w[:, h, :],
                            start=(h == 0), stop=(h == 1))

                # drain dims0-4 bands (fp32 psum -> fp16 sbuf)
                pe_ = {"a": nc.scalar, "v": nc.vector, "p": nc.gpsimd}[PDK_DRAIN[g]]
                if pe_ is nc.scalar:
                    pe_.copy(out=_ap(dks[:, :, :], s0 * 96, [[96, G], [1, 80]]),
                             in_=_ap(pdk[:, :, :], 0, [[128, G], [1, 80]]))
                else:
                    pe_.tensor_copy(
                        out=_ap(dks[:, :, :], s0 * 96, [[96, G], [1, 80]]),
                        in_=_ap(pdk[:, :, :], 0, [[128, G], [1, 80]]))

                # G5 combine: t5 = pg5 * B4, tree-sum over l.
                # GPSIMD cannot touch PSUM on HW: Act stages pg5 to SBUF first.
                t5 = scr.tile([P, G, 4, 16], F16, tag="t5")
                if T5_ENG[g] == "v":
                    t5_in0 = pg5[:, :, :]
                    t5e = nc.vector
                else:
                    pg5s = scr.tile([P, G, 64], F16, tag="pg5s")
                    nc.scalar.copy(out=pg5s[:, :, :], in_=pg5[:, :, :])
                    t5_in0 = pg5s[:, :, :]
                    t5e = nc.gpsimd
                t5e.tensor_tensor(
                    out=t5[:, :, :, :],
                    in0=t5_in0,
                    in1=_ap(Bbig[:, :, :, :], 4 * 4 * S + s0,
                            [[1, G], [S, 4], [0, 16]]),
                    op=MUL)
                u5 = scr.tile([P, G, 2, 16], F16, tag="u5")
                nc.vector.tensor_tensor(
                    out=u5[:, :, :, :],
                    in0=_ap(t5[:, :, :, :], 0, [[64, G], [1, 32]]),
                    in1=_ap(t5[:, :, :, :], 32, [[64, G], [1, 32]]), op=ADD)
                nc.vector.tensor_tensor(
                    out=_ap(dks[:, :, :], s0 * 96 + 80, [[96, G], [1, 16]]),
                    in0=_ap(u5[:, :, :, :], 0, [[32, G], [1, 16]]),
                    in1=_ap(u5[:, :, :, :], 16, [[32, G], [1, 16]]), op=ADD)

                # combine: e1 = dk*px, e2 = e1*pq_rev
                for sb, se in COMBINE_SCHED.get(g, ()):
                    W2 = se - sb
                    e1 = scr.tile([P, 2 * G, 96], F16, tag="e1")
                    nc.vector.tensor_tensor(
                        out=_ap(e1[:, :, :], 0, [[96, W2], [1, 96]]),
                        in0=_ap(dks[:, :, :], sb * 96, [[96, W2], [1, 96]]),
                        in1=_ap(px[:, :, :, :], sb * 96,
                                [[96, W2], [16, 6], [1, 16]]),
                        op=MUL)
                    nc.vector.tensor_tensor(
                        out=_ap(e2[:, :, :], sb * 96, [[96, W2], [1, 96]]),
                        in0=_ap(e1[:, :, :], 0, [[96, W2], [1, 96]]),
                        in1=_ap(pq[:, :, :, :], sb * 96 + 15,
                                [[96, W2], [16, 6], [-1, 16]]),
                        op=MUL)
                # tree-reduce over m + dim product
                for sb, se in TREE_SCHED.get(g, ()):
                    H = se - sb
                    ta = scr.tile([P, 4 * G, 6, 8], F16, tag="ta")
                    nc.vector.tensor_tensor(
                        out=_ap(ta[:, :, :, :], 0, [[48, H], [8, 6], [1, 8]]),
                        in0=_ap(e2[:, :, :], sb * 96, [[96, H], [16, 6], [1, 8]]),
                        in1=_ap(e2[:, :, :], sb * 96 + 8,
                                [[96, H], [16, 6], [1, 8]]), op=ADD)
                    tb = scr.tile([P, 4 * G, 6, 4], F16, tag="tb")
                    nc.vector.tensor_tensor(
                        out=_ap(tb[:, :, :, :], 0, [[24, H], [4, 6], [1, 4]]),
                        in0=_ap(ta[:, :, :, :], 0, [[48, H], [8, 6], [1, 4]]),
                        in1=_ap(ta[:, :, :, :], 4, [[48, H], [8, 6], [1, 4]]),
                        op=ADD)
                    tc = scr.tile([P, 4 * G, 6, 2], F16, tag="tc")
                    nc.vector.tensor_tensor(
                        out=_ap(tc[:, :, :, :], 0, [[12, H], [2, 6], [1, 2]]),
                        in0=_ap(tb[:, :, :, :], 0, [[24, H], [4, 6], [1, 2]]),
                        in1=_ap(tb[:, :, :, :], 2, [[24, H], [4, 6], [1, 2]]),
                        op=ADD)
                    nc.vector.tensor_tensor(
                        out=_ap(s6[:, :, :], sb * DIM, [[DIM, H], [1, DIM]]),
                        in0=_ap(tc[:, :, :, :], 0, [[12, H], [2, DIM]]),
                        in1=_ap(tc[:, :, :, :], 1, [[12, H], [2, DIM]]), op=ADD)
                    # final product over dims for this s-range
                    r1 = scr.tile([P, 4 * G, 3], F16, tag="r1")
                    nc.vector.tensor_tensor(
                        out=_ap(r1[:, :, :], 0, [[3, H], [1, 3]]),
                        in0=_ap(s6[:, :, :], sb * DIM, [[DIM, H], [2, 3]]),
                        in1=_ap(s6[:, :, :], sb * DIM + 1, [[DIM, H], [2, 3]]),
                        op=MUL)
                    r2 = scr.tile([P, 4 * G], F16, tag="r2")
                    nc.vector.tensor_tensor(
                        out=_ap(r2[:, :], 0, [[1, H]]),
                        in0=_ap(r1[:, :, :], 0, [[3, H]]),
                        in1=_ap(r1[:, :, :], 1, [[3, H]]), op=MUL)
                    nc.vector.tensor_tensor(
                        out=_ap(dq[:, :], sb, [[1, H]]),
                        in0=_ap(r2[:, :], 0, [[1, H]]),
                        in1=_ap(r1[:, :, :], 2, [[3, H]]), op=MUL)
                    if DENS_SPLIT:
                        nc.sync.dma_start(
                            out=_ap(dens_out[:, :], sb, [[1, H]]),
                            in_=_ap(dq[:, :], sb, [[1, H]]))

                if FAST_START and g == 0:
                    # remaining slots' prep, overlapped with group-0 pipeline
                    emit_prep(G, S - G)
                    emit_k2_range(G, S - G)
                    emit_k3_range(G, G)
                if C4_EARLY and g == 1:
                    # build late DVE groups' cond4 during the early stall
                    emit_k3_range(32, 16)
                    emit_k3_range(48, 16)
                    for gl in (6, 7):
                        if C4_ENG[gl] == "v":
                            emit_c4(gl)

            if not DENS_SPLIT:
                nc.sync.dma_start(out=dens_out[:, :], in_=dq[:, :])

    nc.finalize()
    return nc


def _softplus64(v):
    return np.logaddexp(0.0, v)


def _host_consts(As):
    """wcomb [128,2,80] f16 and ca5w [128,2,64] f16 from fp64 W'' matrices."""
    kap = 16.0 * np.array([math.comb(15, m) for m in range(16)], dtype=np.float64)
    W = []
    for i in range(DIM):
        c = np.cumsum(_softplus64(As[i].astype(np.float64)), axis=1)
        ca = 2.0 * (1.0 / (1.0 + np.exp(-c)) - 0.5)
        ca_ext = np.concatenate(
            [np.zeros((ca.shape[0], 1)), ca, np.ones((ca.shape[0], 1))], axis=1)
        W.append(kap[None, :] * (ca_ext[:, 1:] - ca_ext[:, :-1]))  # [rows,16]

    wcomb = np.zeros((P, 2, 80), dtype=np.float64)
    for h in range(2):
        for p in range(P):
            c4 = 128 * h + p
            wcomb[p, h, 0:16] = W[0][0]
            wcomb[p, h, 16:32] = W[1][c4 >> 6]
            wcomb[p, h, 32:48] = W[2][c4 >> 4]
            wcomb[p, h, 48:64] = W[3][c4 >> 2]
            wcomb[p, h, 64:80] = W[4][c4]
    ca5w = np.zeros((P, 2, 64), dtype=np.float64)
    for h in range(2):
        for p in range(P):
            for l in range(4):
                ca5w[p, h, l * 16:(l + 1) * 16] = W[5][(128 * h + p) * 4 + l]
    return wcomb.astype(np.float16), ca5w.astype(np.float16)


def kernel(**inputs):
    x = np.asarray(inputs["x"], dtype=np.float32)
    As = [np.asarray(inputs[f"A{i}"], dtype=np.float32) for i in range(DIM)]

    if "nc" not in _CACHE:
        _CACHE["nc"] = _build_nc()
    nc = _CACHE["nc"]

    wcomb, ca5w = _host_consts(As)
    idh = np.eye(P, dtype=np.float16)

    in_maps = []
    for c in range(NCORES):
        xc = x[c * NC:(c + 1) * NC].reshape(P, S, DIM)
        in_maps.append({"xr": xc, "wcomb": wcomb, "ca5w": ca5w, "idh": idh})

    res = run_bass_kernel_spmd(nc, in_maps, core_ids=list(range(NCORES)))
    outs = [r["dens"].reshape(NC) for r in res.results]
    return np.concatenate(outs, axis=0)


if __name__ == "__main__":
    rng = np.random.default_rng(0)
    ins = {"x": rng.uniform(0, 1, (N, DIM)).astype(np.float32)}
    for i in range(DIM):
        ins[f"A{i}"] = rng.uniform(0, 1, ((4 ** i), 15)).astype(np.float32)
    out = kernel(**ins)
    print(out.shape, out[:4])


# revision 3
# speedup vs baseline: 1.1130x; 1.0130x over previous
"""Bernstein flow density kernel v2 for 8x TRN2 NeuronCores.

Math (per sample n):  density(n) = prod_i s_i,
  s_i = sum_{m=0..15} W''_i-contraction:  s_i = sum_m dk_i[n,m] * x_i^m * (1-x_i)^(15-m)
  dk_i[n,:] = cond_i[n,:] @ W''_i,  W''_i[r,m] = kap[m]*(ca_i[r,m]-ca_i[r,m-1])
  (ca[,-1]=0, ca[,15]=1; folds the psi-difference + binomial scaling into W'').

Per core (8192 samples, p-major: n = p*64 + s), all fp16 on-chip except x/psum:
  - DVE: deg-3 tables, Kron chain k2->k3->cond4 (s-innermost, 2x fp16),
    G5 combine (B4-weighted l-sum), combine mults + tree reductions.
  - PE: batched fp16 transposes of cond4 chunks; per-slot matmuls:
    dims0-4 fold into one cond4^T contraction (partition-of-unity marginals
    baked into wcomb rows); dim5 via G5 = cond4^T @ ca5w.
  - Act: PSUM->SBUF drains (fp16 casts).
  - Pool: x^m / (1-x)^m power tables (phi-hat factors).
"""

import math
import sys

import numpy as np

sys.path.insert(0, "/opt/trn_rl_repo")

import concourse.bacc as bacc  # noqa: E402
import concourse.bass as bass  # noqa: E402
import concourse.tile as tile  # noqa: E402
from concourse import mybir  # noqa: E402
from concourse.bass_utils import run_bass_kernel_spmd  # noqa: E402

N = 65536
DIM = 6
NCORES = 8
NC = N // NCORES          # 8192 samples per core
P = 128
S = NC // P               # 64 samples per partition
G = 8                     # slots per group
NG = S // G               # 8 groups

F32 = mybir.dt.float32
F16 = mybir.dt.float16
MUL = mybir.AluOpType.mult
ADD = mybir.AluOpType.add

_CACHE = {}

# per-group engine for the ct4 drain: 'a'=Act, 'v'=DVE, 'p'=Pool
CT4_DRAIN = "aaaaaaaa"
# per-group engine for the pdk drain
PDK_DRAIN = "aaaaaaaa"
# per-group engine for the cond4 build ('v' or 'p')
C4_ENG = "vvppppvv"
# per-group engine for the t5 multiply
T5_ENG = "vvvvvvvv"
FAST_START = True
PTR_BUFS = 2
PDK_BUFS = 1
# g -> list of (slot_begin, slot_end) for e1/e2 combine and trees+product
COMBINE_SCHED = {1: [(0, 16)], 3: [(16, 32)], 5: [(32, 48)], 7: [(48, 64)]}
TREE_SCHED = {3: [(0, 32)], 5: [(32, 48)], 7: [(48, 64)]}
C4_EARLY = True     # build c4 for late DVE-groups during the early stall
CG_BUFS = 4
DENS_SPLIT = True
G0_SPLIT = True
U5D5_ENG = "vvpppppp"   # per-group engine for the l-sum adds (no PSUM: v/p)
M1M2_ENG = "vvvv"       # per combine-batch engine (indexed in order)
TREE_ENG = "vpv"        # per tree-batch engine


def _ap(a, off_elems, dims):
    return bass.AP(tensor=a.tensor, offset=a.offset + off_elems, ap=[a.ap[0]] + dims)


def _build_nc():
    nc = bacc.Bacc(target_bir_lowering=False, trn_type="TRN2")

    xr = nc.dram_tensor("xr", [P, S, DIM], F32, kind="ExternalInput")
    wcomb_d = nc.dram_tensor("wcomb", [P, 2, 80], F16, kind="ExternalInput")
    ca5w_d = nc.dram_tensor("ca5w", [P, 2, 64], F16, kind="ExternalInput")
    idh_d = nc.dram_tensor("idh", [P, P], F16, kind="ExternalInput")
    dens_out = nc.dram_tensor("dens", [P, S], F32, kind="ExternalOutput")

    with tile.TileContext(nc) as tc:
        with (
            tc.tile_pool(name="singles", bufs=1) as sg,
            tc.tile_pool(name="cgp", bufs=CG_BUFS) as cgp,
            tc.tile_pool(name="ct4p", bufs=2) as ct4p,
            tc.tile_pool(name="scr", bufs=2) as scr,
            tc.tile_pool(name="ptr", bufs=PTR_BUFS, space="PSUM") as ptrp,
            tc.tile_pool(name="pg5", bufs=2, space="PSUM") as pg5p,
            tc.tile_pool(name="pdk", bufs=PDK_BUFS, space="PSUM") as pdkp,
        ):
            # ---- inputs / consts ----
            xin = sg.tile([P, S, DIM], F32)
            nc.sync.dma_start(out=xin[:, :, :], in_=xr[:, :, :])
            wcomb = sg.tile([P, 2, 80], F16)
            nc.sync.dma_start(out=wcomb[:, :, :], in_=wcomb_d[:, :, :])
            ca5w = sg.tile([P, 2, 64], F16)
            nc.sync.dma_start(out=ca5w[:, :, :], in_=ca5w_d[:, :, :])
            idh = sg.tile([P, P], F16)
            nc.sync.dma_start(out=idh[:, :], in_=idh_d[:, :])

            # ---- casts ----
            # (s,d)-layout fp16 x and 1-x for power chains (on Act: idle early)
            xh6 = sg.tile([P, S, DIM], F16)
            nc.scalar.copy(out=xh6[:, :, :], in_=xin[:, :, :])
            qh6 = sg.tile([P, S, DIM], F16)
            nc.scalar.activation(out=qh6[:, :, :], in_=xin[:, :, :],
                                 func=mybir.ActivationFunctionType.Copy,
                                 scale=-1.0, bias=1.0)
            # d-major fp16 x / 1-x for deg-3 tables (j = 0..4 only)
            NJ = 5
            xh5 = sg.tile([P, NJ, S], F16)
            qh5 = sg.tile([P, NJ, S], F16)
            x2h = sg.tile([P, NJ, S], F16)
            x3h = sg.tile([P, NJ, S], F16)
            q2h = sg.tile([P, NJ, S], F16)
            q3h = sg.tile([P, NJ, S], F16)
            Bbig = sg.tile([P, NJ, 4, S], F16)

            def emit_prep(sl, w):
                """casts + deg-3 Bernstein tables for slot range [sl, sl+w)."""
                xj = _ap(xin[:, :, :], sl * DIM, [[1, NJ], [DIM, w]])
                d5 = [[S, NJ], [1, w]]
                db = [[4 * S, NJ], [1, w]]
                xs = _ap(xh5[:, :, :], sl, d5)
                qs = _ap(qh5[:, :, :], sl, d5)
                x2s = _ap(x2h[:, :, :], sl, d5)
                q2s = _ap(q2h[:, :, :], sl, d5)
                nc.vector.tensor_copy(out=xs, in_=xj)
                nc.vector.tensor_scalar(out=qs, in0=xj, scalar1=-1.0,
                                        scalar2=1.0, op0=MUL, op1=ADD)
                nc.vector.tensor_tensor(out=x2s, in0=xs, in1=xs, op=MUL)
                nc.vector.tensor_tensor(out=q2s, in0=qs, in1=qs, op=MUL)
                # cubic terms written straight into Bbig (a=3: x^3, a=0: q^3)
                nc.vector.tensor_tensor(
                    out=_ap(Bbig[:, :, :, :], 3 * S + sl, db),
                    in0=x2s, in1=xs, op=MUL)
                nc.vector.tensor_tensor(
                    out=_ap(Bbig[:, :, :, :], 0 * S + sl, db),
                    in0=q2s, in1=qs, op=MUL)
                nc.vector.scalar_tensor_tensor(
                    out=_ap(Bbig[:, :, :, :], 1 * S + sl, db),
                    in0=xs, scalar=3.0, in1=q2s, op0=MUL, op1=MUL)
                nc.vector.scalar_tensor_tensor(
                    out=_ap(Bbig[:, :, :, :], 2 * S + sl, db),
                    in0=x2s, scalar=3.0, in1=qs, op0=MUL, op1=MUL)

            # ---- power chains on Pool: px/pq [p, s, d, m16] ----
            px = sg.tile([P, S, DIM, 16], F16)
            pq = sg.tile([P, S, DIM, 16], F16)
            SD = S * DIM

            def emit_chain(tbl, lvl1):
                t = tbl[:, :, :, :]
                nc.gpsimd.memset(_ap(t, 0, [[16, SD]]), 1.0)
                nc.gpsimd.tensor_copy(out=_ap(t, 1, [[16, SD]]),
                                      in_=lvl1[:, :, :])
                nc.gpsimd.tensor_tensor(
                    out=_ap(t, 2, [[16, SD]]), in0=_ap(t, 1, [[16, SD]]),
                    in1=_ap(t, 1, [[16, SD]]), op=MUL)
                nc.gpsimd.tensor_tensor(
                    out=_ap(t, 3, [[16, SD], [1, 2]]),
                    in0=_ap(t, 1, [[16, SD], [1, 2]]),
                    in1=_ap(t, 2, [[16, SD], [0, 2]]), op=MUL)
                nc.gpsimd.tensor_tensor(
                    out=_ap(t, 5, [[16, SD], [1, 4]]),
                    in0=_ap(t, 1, [[16, SD], [1, 4]]),
                    in1=_ap(t, 4, [[16, SD], [0, 4]]), op=MUL)
                nc.gpsimd.tensor_tensor(
                    out=_ap(t, 9, [[16, SD], [1, 7]]),
                    in0=_ap(t, 1, [[16, SD], [1, 7]]),
                    in1=_ap(t, 8, [[16, SD], [0, 7]]), op=MUL)

            # ---- Kron chain: k2, k3 ----
            k2 = sg.tile([P, 16, S], F16)      # (a0,a1) major, s inner
            k3 = sg.tile([P, 64, S], F16)      # (a0a1a2) major, s inner

            def emit_k2_range(sl, w):
                nc.vector.tensor_tensor(
                    out=_ap(k2[:, :, :], sl, [[4 * S, 4], [S, 4], [1, w]]),
                    in0=_ap(Bbig[:, :, :, :], sl, [[S, 4], [0, 4], [1, w]]),
                    in1=_ap(Bbig[:, :, :, :], 4 * S + sl,
                            [[0, 4], [S, 4], [1, w]]),
                    op=MUL)

            def emit_k3_range(sl, w):
                nc.vector.tensor_tensor(
                    out=_ap(k3[:, :, :], sl, [[4 * S, 16], [S, 4], [1, w]]),
                    in0=_ap(k2[:, :, :], sl, [[S, 16], [0, 4], [1, w]]),
                    in1=_ap(Bbig[:, :, :, :], 2 * 4 * S + sl,
                            [[0, 16], [S, 4], [1, w]]),
                    op=MUL)

            if FAST_START:
                emit_prep(0, G)
                emit_k2_range(0, G)
                emit_k3_range(0, G)
            else:
                emit_prep(0, S)
                emit_k2_range(0, S)
                emit_k3_range(0, 16)

            # ---- per-slot outputs ----
            dks = sg.tile([P, S, 96], F16)     # dk bands: cols 16i+m, i=0..5
            s6 = sg.tile([P, S, DIM], F16)
            e2 = sg.tile([P, S, 96], F16)
            dq = sg.tile([P, S], F32)

            emit_chain(px, xh6)
            emit_chain(pq, qh6)

            cg_tiles = {}

            def emit_c4(g):
                s0 = g * G
                cg = cgp.tile([P, 256, G], F16, tag=f"cg{g % CG_BUFS}")
                eng = nc.vector if C4_ENG[g] == "v" else nc.gpsimd
                eng.tensor_tensor(
                    out=cg[:, :, :],
                    in0=_ap(k3[:, :, :], s0, [[S, 64], [0, 4], [1, G]]),
                    in1=_ap(Bbig[:, :, :, :], 3 * 4 * S + s0,
                            [[0, 64], [S, 4], [1, G]]),
                    op=MUL)
                cg_tiles[g] = cg

            for g in range(NG):
                s0 = g * G
                if C4_EARLY:
                    k3_sched = {1: (16, 16)}
                else:
                    k3_sched = ({1: (16, 16), 2: (32, 16), 3: (48, 16)}
                                if FAST_START else
                                {2: (16, 16), 4: (32, 16), 6: (48, 16)})
                if g in k3_sched:
                    emit_k3_range(*k3_sched[g])
                # cond4 for this group: [p, c256, ds8]
                # first/last groups on DVE (pipeline start + tail), rest Pool
                ptr = ptrp.tile([P, 16, P], F16, tag="ptr")

                def emit_transposes(cg, dl, dh):
                    for ds in range(dl, dh):
                        for h in range(2):
                            nc.tensor.matmul(
                                out=ptr[:, ds * 2 + h, :],
                                lhsT=_ap(cg[:, :, :], h * P * G + ds, [[G, P]]),
                                rhs=idh[:, :],
                                is_transpose=True, start=True, stop=True)

                if g == 0 and FAST_START and G0_SPLIT:
                    # split build+transpose halves for minimal lead-in
                    cg = cgp.tile([P, 256, G], F16, tag="cg0")
                    cg_tiles[0] = cg
                    for hf in range(2):
                        nc.vector.tensor_tensor(
                            out=_ap(cg[:, :, :], hf * 4, [[G, 256], [1, 4]]),
                            in0=_ap(k3[:, :, :], hf * 4,
                                    [[S, 64], [0, 4], [1, 4]]),
                            in1=_ap(Bbig[:, :, :, :], 3 * 4 * S + hf * 4,
                                    [[0, 64], [S, 4], [1, 4]]),
                            op=MUL)
                        emit_transposes(cg, hf * 4, hf * 4 + 4)
                else:
                    if g not in cg_tiles:
                        emit_c4(g)
                    cg = cg_tiles[g]
                    emit_transposes(cg, 0, G)

                # ct4 drain psum -> sbuf, distributed across engines.
                # lowercase = whole drain; split modes use two engines/halves.
                ct4 = ct4p.tile([P, G, 2, P], F16, tag="ct4")
                mode = CT4_DRAIN[g]
                if g == 0 and FAST_START and G0_SPLIT:
                    halves = [("a", 0, 8), ("a", 8, 8)]
                else:
                    halves = {"a": [("a", 0, 16)], "v": [("v", 0, 16)],
                              "p": [("p", 0, 16)],
                              "s": [("a", 0, 8), ("v", 8, 8)],
                              "t": [("a", 0, 8), ("p", 8, 8)],
                              "u": [("v", 0, 8), ("p", 8, 8)]}[mode]
                for engc, off, w in halves:
                    de = {"a": nc.scalar, "v": nc.vector, "p": nc.gpsimd}[engc]
                    oap = _ap(ct4[:, :, :, :], off * P, [[1, w * P]])
                    iap = _ap(ptr[:, :, :], off * P, [[1, w * P]])
                    if de is nc.scalar:
                        de.copy(out=oap, in_=iap)
                    else:
                        de.tensor_copy(out=oap, in_=iap)

                # matmuls per slot: dims0-4 -> pdk, dim5 G5 -> pg5
                pdk = pdkp.tile([P, G, 128], F32, tag="pdk")
                pg5 = pg5p.tile([P, G, 64], F32, tag="pg5")
                for ds in range(G):
                    for h in range(2):
                        nc.tensor.matmul(
                            out=_ap(pdk[:, :, :], ds * 128, [[1, 80]]),
                            lhsT=ct4[:, ds, h, :], rhs=wcomb[:, h, :],
                            start=(h == 0), stop=(h == 1))
                    for h in range(2):
                        nc.tensor.matmul(
                            out=pg5[:, ds, :],
                            lhsT=ct4[:, ds, h, :], rhs=ca5w[:, h, :],
                            start=(h == 0), stop=(h == 1))

                # drain dims0-4 bands (fp32 psum -> fp16 sbuf)
                pe_ = {"a": nc.scalar, "v": nc.vector, "p": nc.gpsimd}[PDK_DRAIN[g]]
                if pe_ is nc.scalar:
                    pe_.copy(out=_ap(dks[:, :, :], s0 * 96, [[96, G], [1, 80]]),
                             in_=_ap(pdk[:, :, :], 0, [[128, G], [1, 80]]))
                else:
                    pe_.tensor_copy(
                        out=_ap(dks[:, :, :], s0 * 96, [[96, G], [1, 80]]),
                        in_=_ap(pdk[:, :, :], 0, [[128, G], [1, 80]]))

                # G5 combine: t5 = pg5 * B4, tree-sum over l.
                # GPSIMD cannot touch PSUM on HW: Act stages pg5 to SBUF first.
                t5 = scr.tile([P, G, 4, 16], F16, tag="t5")
                if T5_ENG[g] == "v":
                    t5_in0 = pg5[:, :, :]
                    t5e = nc.vector
                else:
                    pg5s = scr.tile([P, G, 64], F16, tag="pg5s")
                    nc.scalar.copy(out=pg5s[:, :, :], in_=pg5[:, :, :])
                    t5_in0 = pg5s[:, :, :]
                    t5e = nc.gpsimd
                t5e.tensor_tensor(
                    out=t5[:, :, :, :],
                    in0=t5_in0,
                    in1=_ap(Bbig[:, :, :, :], 4 * 4 * S + s0,
                            [[1, G], [S, 4], [0, 16]]),
                    op=MUL)
                u5 = scr.tile([P, G, 2, 16], F16, tag="u5")
                u5e = nc.vector if U5D5_ENG[g] == "v" else nc.gpsimd
                u5e.tensor_tensor(
                    out=u5[:, :, :, :],
                    in0=_ap(t5[:, :, :, :], 0, [[64, G], [1, 32]]),
                    in1=_ap(t5[:, :, :, :], 32, [[64, G], [1, 32]]), op=ADD)
                u5e.tensor_tensor(
                    out=_ap(dks[:, :, :], s0 * 96 + 80, [[96, G], [1, 16]]),
                    in0=_ap(u5[:, :, :, :], 0, [[32, G], [1, 16]]),
                    in1=_ap(u5[:, :, :, :], 16, [[32, G], [1, 16]]), op=ADD)

                # combine: e1 = dk*px, e2 = e1*pq_rev
                for ci, (sb, se) in enumerate(COMBINE_SCHED.get(g, ())):
                    cidx = sum(len(v) for k, v in COMBINE_SCHED.items()
                               if k < g) + ci
                    me = nc.vector if M1M2_ENG[cidx] == "v" else nc.gpsimd
                    W2 = se - sb
                    e1 = scr.tile([P, 2 * G, 96], F16, tag="e1")
                    me.tensor_tensor(
                        out=_ap(e1[:, :, :], 0, [[96, W2], [1, 96]]),
                        in0=_ap(dks[:, :, :], sb * 96, [[96, W2], [1, 96]]),
                        in1=_ap(px[:, :, :, :], sb * 96,
                                [[96, W2], [16, 6], [1, 16]]),
                        op=MUL)
                    me.tensor_tensor(
                        out=_ap(e2[:, :, :], sb * 96, [[96, W2], [1, 96]]),
                        in0=_ap(e1[:, :, :], 0, [[96, W2], [1, 96]]),
                        in1=_ap(pq[:, :, :, :], sb * 96 + 15,
                                [[96, W2], [16, 6], [-1, 16]]),
                        op=MUL)
                # tree-reduce over m + dim product
                for ti, (sb, se) in enumerate(TREE_SCHED.get(g, ())):
                    tidx = sum(len(v) for k, v in TREE_SCHED.items()
                               if k < g) + ti
                    te_ = nc.vector if TREE_ENG[tidx] == "v" else nc.gpsimd
                    H = se - sb
                    ta = scr.tile([P, 4 * G, 6, 8], F16, tag="ta")
                    te_.tensor_tensor(
                        out=_ap(ta[:, :, :, :], 0, [[48, H], [8, 6], [1, 8]]),
                        in0=_ap(e2[:, :, :], sb * 96, [[96, H], [16, 6], [1, 8]]),
                        in1=_ap(e2[:, :, :], sb * 96 + 8,
                                [[96, H], [16, 6], [1, 8]]), op=ADD)
                    tb = scr.tile([P, 4 * G, 6, 4], F16, tag="tb")
                    te_.tensor_tensor(
                        out=_ap(tb[:, :, :, :], 0, [[24, H], [4, 6], [1, 4]]),
                        in0=_ap(ta[:, :, :, :], 0, [[48, H], [8, 6], [1, 4]]),
                        in1=_ap(ta[:, :, :, :], 4, [[48, H], [8, 6], [1, 4]]),
                        op=ADD)
                    tc = scr.tile([P, 4 * G, 6, 2], F16, tag="tc")
                    te_.tensor_tensor(
                        out=_ap(tc[:, :, :, :], 0, [[12, H], [2, 6], [1, 2]]),
                        in0=_ap(tb[:, :, :, :], 0, [[24, H], [4, 6], [1, 2]]),
                        in1=_ap(tb[:, :, :, :], 2, [[24, H], [4, 6], [1, 2]]),
                        op=ADD)
                    te_.tensor_tensor(
                        out=_ap(s6[:, :, :], sb * DIM, [[DIM, H], [1, DIM]]),
                        in0=_ap(tc[:, :, :, :], 0, [[12, H], [2, DIM]]),
                        in1=_ap(tc[:, :, :, :], 1, [[12, H], [2, DIM]]), op=ADD)
                    # final product over dims for this s-range
                    r1 = scr.tile([P, 4 * G, 3], F16, tag="r1")
                    te_.tensor_tensor(
                        out=_ap(r1[:, :, :], 0, [[3, H], [1, 3]]),
                        in0=_ap(s6[:, :, :], sb * DIM, [[DIM, H], [2, 3]]),
                        in1=_ap(s6[:, :, :], sb * DIM + 1, [[DIM, H], [2, 3]]),
                        op=MUL)
                    r2 = scr.tile([P, 4 * G], F16, tag="r2")
                    te_.tensor_tensor(
                        out=_ap(r2[:, :], 0, [[1, H]]),
                        in0=_ap(r1[:, :, :], 0, [[3, H]]),
                        in1=_ap(r1[:, :, :], 1, [[3, H]]), op=MUL)
                    te_.tensor_tensor(
                        out=_ap(dq[:, :], sb, [[1, H]]),
                        in0=_ap(r2[:, :], 0, [[1, H]]),
                        in1=_ap(r1[:, :, :], 2, [[3, H]]), op=MUL)
                    if DENS_SPLIT:
                        nc.sync.dma_start(
                            out=_ap(dens_out[:, :], sb, [[1, H]]),
                            in_=_ap(dq[:, :], sb, [[1, H]]))

                if FAST_START and g == 0:
                    # remaining slots' prep, overlapped with group-0 pipeline
                    emit_prep(G, S - G)
                    emit_k2_range(G, S - G)
                    emit_k3_range(G, G)
                if C4_EARLY and g == 1:
                    # build late DVE groups' cond4 during the early stall
                    emit_k3_range(32, 16)
                    emit_k3_range(48, 16)
                    for gl in (6, 7):
                        if C4_ENG[gl] == "v":
                            emit_c4(gl)

            if not DENS_SPLIT:
                nc.sync.dma_start(out=dens_out[:, :], in_=dq[:, :])

    nc.finalize()
    return nc


def _softplus64(v):
    return np.logaddexp(0.0, v)


def _host_consts(As):
    """wcomb [128,2,80] f16 and ca5w [128,2,64] f16 from fp64 W'' matrices."""
    kap = 16.0 * np.array([math.comb(15, m) for m in range(16)], dtype=np.float64)
    W = []
    for i in range(DIM):
        c = np.cumsum(_softplus64(As[i].astype(np.float64)), axis=1)
        ca = 2.0 * (1.0 / (1.0 + np.exp(-c)) - 0.5)
        ca_ext = np.concatenate(
            [np.zeros((ca.shape[0], 1)), ca, np.ones((ca.shape[0], 1))], axis=1)
        W.append(kap[None, :] * (ca_ext[:, 1:] - ca_ext[:, :-1]))  # [rows,16]

    wcomb = np.zeros((P, 2, 80), dtype=np.float64)
    for h in range(2):
        for p in range(P):
            c4 = 128 * h + p
            wcomb[p, h, 0:16] = W[0][0]
            wcomb[p, h, 16:32] = W[1][c4 >> 6]
            wcomb[p, h, 32:48] = W[2][c4 >> 4]
            wcomb[p, h, 48:64] = W[3][c4 >> 2]
            wcomb[p, h, 64:80] = W[4][c4]
    ca5w = np.zeros((P, 2, 64), dtype=np.float64)
    for h in range(2):
        for p in range(P):
            for l in range(4):
                ca5w[p, h, l * 16:(l + 1) * 16] = W[5][(128 * h + p) * 4 + l]
    return wcomb.astype(np.float16), ca5w.astype(np.float16)


def kernel(**inputs):
    x = np.asarray(inputs["x"], dtype=np.float32)
    As = [np.asarray(inputs[f"A{i}"], dtype=np.float32) for i in range(DIM)]

    if "nc" not in _CACHE:
        _CACHE["nc"] = _build_nc()
    nc = _CACHE["nc"]

    wcomb, ca5w = _host_consts(As)
    idh = np.eye(P, dtype=np.float16)

    in_maps = []
    for c in range(NCORES):
        xc = x[c * NC:(c + 1) * NC].reshape(P, S, DIM)
        in_maps.append({"xr": xc, "wcomb": wcomb, "ca5w": ca5w, "idh": idh})

    res = run_bass_kernel_spmd(nc, in_maps, core_ids=list(range(NCORES)))
    outs = [r["dens"].reshape(NC) for r in res.results]
    return np.concatenate(outs, axis=0)


if __name__ == "__main__":
    rng = np.random.default_rng(0)
    ins = {"x": rng.uniform(0, 1, (N, DIM)).astype(np.float32)}
    for i in range(DIM):
        ins[f"A{i}"] = rng.uniform(0, 1, ((4 ** i), 15)).astype(np.float32)
    out = kernel(**ins)
    print(out.shape, out[:4])


# revision 4
# speedup vs baseline: 1.1348x; 1.0196x over previous
"""Bernstein flow density kernel v2 for 8x TRN2 NeuronCores.

Math (per sample n):  density(n) = prod_i s_i,
  s_i = sum_{m=0..15} W''_i-contraction:  s_i = sum_m dk_i[n,m] * x_i^m * (1-x_i)^(15-m)
  dk_i[n,:] = cond_i[n,:] @ W''_i,  W''_i[r,m] = kap[m]*(ca_i[r,m]-ca_i[r,m-1])
  (ca[,-1]=0, ca[,15]=1; folds the psi-difference + binomial scaling into W'').

Per core (8192 samples, p-major: n = p*64 + s), all fp16 on-chip except x/psum:
  - DVE: deg-3 tables, Kron chain k2->k3->cond4 (s-innermost, 2x fp16),
    G5 combine (B4-weighted l-sum), combine mults + tree reductions.
  - PE: batched fp16 transposes of cond4 chunks; per-slot matmuls:
    dims0-4 fold into one cond4^T contraction (partition-of-unity marginals
    baked into wcomb rows); dim5 via G5 = cond4^T @ ca5w.
  - Act: PSUM->SBUF drains (fp16 casts).
  - Pool: x^m / (1-x)^m power tables (phi-hat factors).
"""

import math
import sys

import numpy as np

sys.path.insert(0, "/opt/trn_rl_repo")

import concourse.bacc as bacc  # noqa: E402
import concourse.bass as bass  # noqa: E402
import concourse.tile as tile  # noqa: E402
from concourse import mybir  # noqa: E402
from concourse.bass_utils import run_bass_kernel_spmd  # noqa: E402

N = 65536
DIM = 6
NCORES = 8
NC = N // NCORES          # 8192 samples per core
P = 128
S = NC // P               # 64 samples per partition
G = 8                     # slots per group
NG = S // G               # groups

F32 = mybir.dt.float32
F16 = mybir.dt.float16
MUL = mybir.AluOpType.mult
ADD = mybir.AluOpType.add

_CACHE = {}

def _expand(s8):
    """scale an 8-char per-group config string to NG groups"""
    return "".join(s8[g * 8 // NG] for g in range(NG))

# per-group engine for the ct4 drain: 'a'=Act, 'v'=DVE, 'p'=Pool
CT4_DRAIN = _expand("aaaaaaaa")
# per-group engine for the pdk drain
PDK_DRAIN = _expand("aaaaaaaa")
# per-group engine for the cond4 build ('v' or 'p')
C4_ENG = _expand("vvppppvv")
# per-group engine for the t5 multiply
T5_ENG = _expand("vvvvvvvv")
FAST_START = True
PTR_BUFS = 2
PDK_BUFS = 1
# combine after every 16 slots; trees in 3 batches (32/16/16)
COMBINE_SCHED = {16 // G - 1: [(0, 16)], 32 // G - 1: [(16, 32)],
                 48 // G - 1: [(32, 48)],
                 64 // G - 1: [(48, 56), (56, 64)]}
TREE_SCHED = {32 // G - 1: [(0, 32)], 48 // G - 1: [(32, 48)],
              64 // G - 1: [(48, 56), (56, 64)]}
C4_EARLY = True     # build c4 for late DVE-groups during the early stall
CG_BUFS = 4
DENS_SPLIT = True
CT4P_BUFS = 2
SCR_BUFS = 2
G0_SPLIT = True
U5D5_ENG = _expand("vvvvvvvv")  # per-group engine for the l-sum adds
M1M2_ENG = "vvvvp"      # per combine-batch engine (indexed in order)
TREE_ENG = "vppv"       # per tree-batch engine


def _ap(a, off_elems, dims):
    return bass.AP(tensor=a.tensor, offset=a.offset + off_elems, ap=[a.ap[0]] + dims)


def _build_nc():
    nc = bacc.Bacc(target_bir_lowering=False, trn_type="TRN2")

    xr = nc.dram_tensor("xr", [P, S, DIM], F32, kind="ExternalInput")
    wcomb_d = nc.dram_tensor("wcomb", [P, 2, 80], F16, kind="ExternalInput")
    ca5w_d = nc.dram_tensor("ca5w", [P, 2, 64], F16, kind="ExternalInput")
    idh_d = nc.dram_tensor("idh", [P, P], F16, kind="ExternalInput")
    dens_out = nc.dram_tensor("dens", [P, S], F32, kind="ExternalOutput")

    with tile.TileContext(nc) as tc:
        with (
            tc.tile_pool(name="singles", bufs=1) as sg,
            tc.tile_pool(name="cgp", bufs=CG_BUFS) as cgp,
            tc.tile_pool(name="ct4p", bufs=CT4P_BUFS) as ct4p,
            tc.tile_pool(name="scr", bufs=SCR_BUFS) as scr,
            tc.tile_pool(name="ptr", bufs=PTR_BUFS, space="PSUM") as ptrp,
            tc.tile_pool(name="pg5", bufs=2, space="PSUM") as pg5p,
            tc.tile_pool(name="pdk", bufs=PDK_BUFS, space="PSUM") as pdkp,
        ):
            # ---- inputs / consts ----
            xin = sg.tile([P, S, DIM], F32)
            nc.sync.dma_start(out=xin[:, :, :], in_=xr[:, :, :])
            wcomb = sg.tile([P, 2, 80], F16)
            nc.sync.dma_start(out=wcomb[:, :, :], in_=wcomb_d[:, :, :])
            ca5w = sg.tile([P, 2, 64], F16)
            nc.sync.dma_start(out=ca5w[:, :, :], in_=ca5w_d[:, :, :])
            idh = sg.tile([P, P], F16)
            nc.sync.dma_start(out=idh[:, :], in_=idh_d[:, :])

            # ---- casts ----
            # (s,d)-layout fp16 x and 1-x for power chains (on Act: idle early)
            xh6 = sg.tile([P, S, DIM], F16)
            nc.scalar.copy(out=xh6[:, :, :], in_=xin[:, :, :])
            qh6 = sg.tile([P, S, DIM], F16)
            nc.scalar.activation(out=qh6[:, :, :], in_=xin[:, :, :],
                                 func=mybir.ActivationFunctionType.Copy,
                                 scale=-1.0, bias=1.0)
            # d-major fp16 x / 1-x for deg-3 tables (j = 0..4 only)
            NJ = 5
            xh5 = sg.tile([P, NJ, S], F16)
            qh5 = sg.tile([P, NJ, S], F16)
            x2h = sg.tile([P, NJ, S], F16)
            x3h = sg.tile([P, NJ, S], F16)
            q2h = sg.tile([P, NJ, S], F16)
            q3h = sg.tile([P, NJ, S], F16)
            Bbig = sg.tile([P, NJ, 4, S], F16)

            def emit_prep(sl, w):
                """casts + deg-3 Bernstein tables for slot range [sl, sl+w)."""
                xj = _ap(xin[:, :, :], sl * DIM, [[1, NJ], [DIM, w]])
                d5 = [[S, NJ], [1, w]]
                db = [[4 * S, NJ], [1, w]]
                xs = _ap(xh5[:, :, :], sl, d5)
                qs = _ap(qh5[:, :, :], sl, d5)
                x2s = _ap(x2h[:, :, :], sl, d5)
                q2s = _ap(q2h[:, :, :], sl, d5)
                nc.vector.tensor_copy(out=xs, in_=xj)
                nc.vector.tensor_scalar(out=qs, in0=xj, scalar1=-1.0,
                                        scalar2=1.0, op0=MUL, op1=ADD)
                nc.vector.tensor_tensor(out=x2s, in0=xs, in1=xs, op=MUL)
                nc.vector.tensor_tensor(out=q2s, in0=qs, in1=qs, op=MUL)
                # cubic terms written straight into Bbig (a=3: x^3, a=0: q^3)
                nc.vector.tensor_tensor(
                    out=_ap(Bbig[:, :, :, :], 3 * S + sl, db),
                    in0=x2s, in1=xs, op=MUL)
                nc.vector.tensor_tensor(
                    out=_ap(Bbig[:, :, :, :], 0 * S + sl, db),
                    in0=q2s, in1=qs, op=MUL)
                nc.vector.scalar_tensor_tensor(
                    out=_ap(Bbig[:, :, :, :], 1 * S + sl, db),
                    in0=xs, scalar=3.0, in1=q2s, op0=MUL, op1=MUL)
                nc.vector.scalar_tensor_tensor(
                    out=_ap(Bbig[:, :, :, :], 2 * S + sl, db),
                    in0=x2s, scalar=3.0, in1=qs, op0=MUL, op1=MUL)

            # ---- power chains on Pool: px/pq [p, s, d, m16] ----
            px = sg.tile([P, S, DIM, 16], F16)
            pq = sg.tile([P, S, DIM, 16], F16)
            SD = S * DIM

            def emit_chain(tbl, lvl1):
                t = tbl[:, :, :, :]
                nc.gpsimd.memset(_ap(t, 0, [[16, SD]]), 1.0)
                nc.gpsimd.tensor_copy(out=_ap(t, 1, [[16, SD]]),
                                      in_=lvl1[:, :, :])
                nc.gpsimd.tensor_tensor(
                    out=_ap(t, 2, [[16, SD]]), in0=_ap(t, 1, [[16, SD]]),
                    in1=_ap(t, 1, [[16, SD]]), op=MUL)
                nc.gpsimd.tensor_tensor(
                    out=_ap(t, 3, [[16, SD], [1, 2]]),
                    in0=_ap(t, 1, [[16, SD], [1, 2]]),
                    in1=_ap(t, 2, [[16, SD], [0, 2]]), op=MUL)
                nc.gpsimd.tensor_tensor(
                    out=_ap(t, 5, [[16, SD], [1, 4]]),
                    in0=_ap(t, 1, [[16, SD], [1, 4]]),
                    in1=_ap(t, 4, [[16, SD], [0, 4]]), op=MUL)
                nc.gpsimd.tensor_tensor(
                    out=_ap(t, 9, [[16, SD], [1, 7]]),
                    in0=_ap(t, 1, [[16, SD], [1, 7]]),
                    in1=_ap(t, 8, [[16, SD], [0, 7]]), op=MUL)

            # ---- Kron chain: k2, k3 ----
            k2 = sg.tile([P, 16, S], F16)      # (a0,a1) major, s inner
            k3 = sg.tile([P, 64, S], F16)      # (a0a1a2) major, s inner

            def emit_k2_range(sl, w):
                nc.vector.tensor_tensor(
                    out=_ap(k2[:, :, :], sl, [[4 * S, 4], [S, 4], [1, w]]),
                    in0=_ap(Bbig[:, :, :, :], sl, [[S, 4], [0, 4], [1, w]]),
                    in1=_ap(Bbig[:, :, :, :], 4 * S + sl,
                            [[0, 4], [S, 4], [1, w]]),
                    op=MUL)

            def emit_k3_range(sl, w):
                nc.vector.tensor_tensor(
                    out=_ap(k3[:, :, :], sl, [[4 * S, 16], [S, 4], [1, w]]),
                    in0=_ap(k2[:, :, :], sl, [[S, 16], [0, 4], [1, w]]),
                    in1=_ap(Bbig[:, :, :, :], 2 * 4 * S + sl,
                            [[0, 16], [S, 4], [1, w]]),
                    op=MUL)

            if FAST_START:
                emit_prep(0, G)
                emit_k2_range(0, G)
                emit_k3_range(0, G)
            else:
                emit_prep(0, S)
                emit_k2_range(0, S)
                emit_k3_range(0, 16)

            # ---- per-slot outputs ----
            dks = sg.tile([P, S, 96], F16)     # dk bands: cols 16i+m, i=0..5
            s6 = sg.tile([P, S, DIM], F16)
            e2 = sg.tile([P, S, 96], F16)
            dq = sg.tile([P, S], F32)

            emit_chain(px, xh6)
            emit_chain(pq, qh6)

            cg_tiles = {}

            def emit_c4(g):
                s0 = g * G
                cg = cgp.tile([P, 256, G], F16, tag=f"cg{g % CG_BUFS}")
                eng = nc.vector if C4_ENG[g] == "v" else nc.gpsimd
                eng.tensor_tensor(
                    out=cg[:, :, :],
                    in0=_ap(k3[:, :, :], s0, [[S, 64], [0, 4], [1, G]]),
                    in1=_ap(Bbig[:, :, :, :], 3 * 4 * S + s0,
                            [[0, 64], [S, 4], [1, G]]),
                    op=MUL)
                cg_tiles[g] = cg

            for g in range(NG):
                s0 = g * G
                if C4_EARLY:
                    k3_sched = {max(1, 16 // G - 2): (16, 16)}
                else:
                    k3_sched = {max(1, 16 // G - 2): (16, 16),
                                max(2, 32 // G - 2): (32, 16),
                                max(3, 48 // G - 2): (48, 16)}
                if g in k3_sched:
                    emit_k3_range(*k3_sched[g])
                # cond4 for this group: [p, c256, ds8]
                # first/last groups on DVE (pipeline start + tail), rest Pool
                ptr = ptrp.tile([P, 2 * G, P], F16, tag="ptr")

                def emit_transposes(cg, dl, dh):
                    for ds in range(dl, dh):
                        for h in range(2):
                            nc.tensor.matmul(
                                out=ptr[:, ds * 2 + h, :],
                                lhsT=_ap(cg[:, :, :], h * P * G + ds, [[G, P]]),
                                rhs=idh[:, :],
                                is_transpose=True, start=True, stop=True)

                if g == 0 and FAST_START and G0_SPLIT:
                    # split build+transpose halves for minimal lead-in
                    cg = cgp.tile([P, 256, G], F16, tag="cg0")
                    cg_tiles[0] = cg
                    GH = G // 2
                    for hf in range(2):
                        nc.vector.tensor_tensor(
                            out=_ap(cg[:, :, :], hf * GH, [[G, 256], [1, GH]]),
                            in0=_ap(k3[:, :, :], hf * GH,
                                    [[S, 64], [0, 4], [1, GH]]),
                            in1=_ap(Bbig[:, :, :, :], 3 * 4 * S + hf * GH,
                                    [[0, 64], [S, 4], [1, GH]]),
                            op=MUL)
                        emit_transposes(cg, hf * GH, hf * GH + GH)
                else:
                    if g not in cg_tiles:
                        emit_c4(g)
                    cg = cg_tiles[g]
                    emit_transposes(cg, 0, G)

                # ct4 drain psum -> sbuf, distributed across engines.
                # lowercase = whole drain; split modes use two engines/halves.
                ct4 = ct4p.tile([P, G, 2, P], F16, tag="ct4")
                mode = CT4_DRAIN[g]
                if g == 0 and FAST_START and G0_SPLIT:
                    halves = [("a", 0, G), ("a", G, G)]
                else:
                    G2 = 2 * G
                    halves = {"a": [("a", 0, G2)], "v": [("v", 0, G2)],
                              "p": [("p", 0, G2)],
                              "s": [("a", 0, G), ("v", G, G)],
                              "t": [("a", 0, G), ("p", G, G)],
                              "u": [("v", 0, G), ("p", G, G)]}[mode]
                for engc, off, w in halves:
                    de = {"a": nc.scalar, "v": nc.vector, "p": nc.gpsimd}[engc]
                    oap = _ap(ct4[:, :, :, :], off * P, [[1, w * P]])
                    iap = _ap(ptr[:, :, :], off * P, [[1, w * P]])
                    if de is nc.scalar:
                        de.copy(out=oap, in_=iap)
                    else:
                        de.tensor_copy(out=oap, in_=iap)

                # matmuls per slot: dims0-4 -> pdk, dim5 G5 -> pg5
                pdk = pdkp.tile([P, G, 128], F32, tag="pdk")
                pg5 = pg5p.tile([P, G, 64], F32, tag="pg5")
                for ds in range(G):
                    for h in range(2):
                        nc.tensor.matmul(
                            out=_ap(pdk[:, :, :], ds * 128, [[1, 80]]),
                            lhsT=ct4[:, ds, h, :], rhs=wcomb[:, h, :],
                            start=(h == 0), stop=(h == 1))
                    for h in range(2):
                        nc.tensor.matmul(
                            out=pg5[:, ds, :],
                            lhsT=ct4[:, ds, h, :], rhs=ca5w[:, h, :],
                            start=(h == 0), stop=(h == 1))

                # drain dims0-4 bands (fp32 psum -> fp16 sbuf)
                pe_ = {"a": nc.scalar, "v": nc.vector, "p": nc.gpsimd}[PDK_DRAIN[g]]
                if pe_ is nc.scalar:
                    pe_.copy(out=_ap(dks[:, :, :], s0 * 96, [[96, G], [1, 80]]),
                             in_=_ap(pdk[:, :, :], 0, [[128, G], [1, 80]]))
                else:
                    pe_.tensor_copy(
                        out=_ap(dks[:, :, :], s0 * 96, [[96, G], [1, 80]]),
                        in_=_ap(pdk[:, :, :], 0, [[128, G], [1, 80]]))

                # G5 combine: t5 = pg5 * B4, tree-sum over l.
                # GPSIMD cannot touch PSUM on HW: Act stages pg5 to SBUF first.
                t5 = scr.tile([P, G, 4, 16], F16, tag="t5")
                if T5_ENG[g] == "v":
                    t5_in0 = pg5[:, :, :]
                    t5e = nc.vector
                else:
                    pg5s = scr.tile([P, G, 64], F16, tag="pg5s")
                    nc.scalar.copy(out=pg5s[:, :, :], in_=pg5[:, :, :])
                    t5_in0 = pg5s[:, :, :]
                    t5e = nc.gpsimd
                t5e.tensor_tensor(
                    out=t5[:, :, :, :],
                    in0=t5_in0,
                    in1=_ap(Bbig[:, :, :, :], 4 * 4 * S + s0,
                            [[1, G], [S, 4], [0, 16]]),
                    op=MUL)
                u5 = scr.tile([P, G, 2, 16], F16, tag="u5")
                u5e = nc.vector if U5D5_ENG[g] == "v" else nc.gpsimd
                u5e.tensor_tensor(
                    out=u5[:, :, :, :],
                    in0=_ap(t5[:, :, :, :], 0, [[64, G], [1, 32]]),
                    in1=_ap(t5[:, :, :, :], 32, [[64, G], [1, 32]]), op=ADD)
                u5e.tensor_tensor(
                    out=_ap(dks[:, :, :], s0 * 96 + 80, [[96, G], [1, 16]]),
                    in0=_ap(u5[:, :, :, :], 0, [[32, G], [1, 16]]),
                    in1=_ap(u5[:, :, :, :], 16, [[32, G], [1, 16]]), op=ADD)

                # combine: e1 = dk*px, e2 = e1*pq_rev
                for ci, (sb, se) in enumerate(COMBINE_SCHED.get(g, ())):
                    cidx = sum(len(v) for k, v in COMBINE_SCHED.items()
                               if k < g) + ci
                    me = nc.vector if M1M2_ENG[cidx] == "v" else nc.gpsimd
                    W2 = se - sb
                    e1 = scr.tile([P, 16, 96], F16, tag="e1")
                    me.tensor_tensor(
                        out=_ap(e1[:, :, :], 0, [[96, W2], [1, 96]]),
                        in0=_ap(dks[:, :, :], sb * 96, [[96, W2], [1, 96]]),
                        in1=_ap(px[:, :, :, :], sb * 96,
                                [[96, W2], [16, 6], [1, 16]]),
                        op=MUL)
                    me.tensor_tensor(
                        out=_ap(e2[:, :, :], sb * 96, [[96, W2], [1, 96]]),
                        in0=_ap(e1[:, :, :], 0, [[96, W2], [1, 96]]),
                        in1=_ap(pq[:, :, :, :], sb * 96 + 15,
                                [[96, W2], [16, 6], [-1, 16]]),
                        op=MUL)
                # tree-reduce over m + dim product
                for ti, (sb, se) in enumerate(TREE_SCHED.get(g, ())):
                    tidx = sum(len(v) for k, v in TREE_SCHED.items()
                               if k < g) + ti
                    te_ = nc.vector if TREE_ENG[tidx] == "v" else nc.gpsimd
                    H = se - sb
                    ta = scr.tile([P, 32, 6, 8], F16, tag="ta")
                    te_.tensor_tensor(
                        out=_ap(ta[:, :, :, :], 0, [[48, H], [8, 6], [1, 8]]),
                        in0=_ap(e2[:, :, :], sb * 96, [[96, H], [16, 6], [1, 8]]),
                        in1=_ap(e2[:, :, :], sb * 96 + 8,
                                [[96, H], [16, 6], [1, 8]]), op=ADD)
                    tb = scr.tile([P, 32, 6, 4], F16, tag="tb")
                    te_.tensor_tensor(
                        out=_ap(tb[:, :, :, :], 0, [[24, H], [4, 6], [1, 4]]),
                        in0=_ap(ta[:, :, :, :], 0, [[48, H], [8, 6], [1, 4]]),
                        in1=_ap(ta[:, :, :, :], 4, [[48, H], [8, 6], [1, 4]]),
                        op=ADD)
                    tc = scr.tile([P, 32, 6, 2], F16, tag="tc")
                    te_.tensor_tensor(
                        out=_ap(tc[:, :, :, :], 0, [[12, H], [2, 6], [1, 2]]),
                        in0=_ap(tb[:, :, :, :], 0, [[24, H], [4, 6], [1, 2]]),
                        in1=_ap(tb[:, :, :, :], 2, [[24, H], [4, 6], [1, 2]]),
                        op=ADD)
                    te_.tensor_tensor(
                        out=_ap(s6[:, :, :], sb * DIM, [[DIM, H], [1, DIM]]),
                        in0=_ap(tc[:, :, :, :], 0, [[12, H], [2, DIM]]),
                        in1=_ap(tc[:, :, :, :], 1, [[12, H], [2, DIM]]), op=ADD)
                    # final product over dims for this s-range
                    r1 = scr.tile([P, 32, 3], F16, tag="r1")
                    te_.tensor_tensor(
                        out=_ap(r1[:, :, :], 0, [[3, H], [1, 3]]),
                        in0=_ap(s6[:, :, :], sb * DIM, [[DIM, H], [2, 3]]),
                        in1=_ap(s6[:, :, :], sb * DIM + 1, [[DIM, H], [2, 3]]),
                        op=MUL)
                    r2 = scr.tile([P, 32], F16, tag="r2")
                    te_.tensor_tensor(
                        out=_ap(r2[:, :], 0, [[1, H]]),
                        in0=_ap(r1[:, :, :], 0, [[3, H]]),
                        in1=_ap(r1[:, :, :], 1, [[3, H]]), op=MUL)
                    te_.tensor_tensor(
                        out=_ap(dq[:, :], sb, [[1, H]]),
                        in0=_ap(r2[:, :], 0, [[1, H]]),
                        in1=_ap(r1[:, :, :], 2, [[3, H]]), op=MUL)
                    if DENS_SPLIT:
                        nc.sync.dma_start(
                            out=_ap(dens_out[:, :], sb, [[1, H]]),
                            in_=_ap(dq[:, :], sb, [[1, H]]))

                if FAST_START and g == 0:
                    # remaining slots' prep, overlapped with group-0 pipeline
                    emit_prep(G, S - G)
                    emit_k2_range(G, S - G)
                    emit_k3_range(G, 16 - G)
                if C4_EARLY and g == 1:
                    # build late DVE groups' cond4 during the early stall
                    emit_k3_range(32, 16)
                    emit_k3_range(48, 16)
                    for gl in range(NG - 16 // G, NG):
                        if C4_ENG[gl] == "v":
                            emit_c4(gl)

            if not DENS_SPLIT:
                nc.sync.dma_start(out=dens_out[:, :], in_=dq[:, :])

    nc.finalize()
    return nc


def _softplus64(v):
    return np.logaddexp(0.0, v)


def _host_consts(As):
    """wcomb [128,2,80] f16 and ca5w [128,2,64] f16 from fp64 W'' matrices."""
    kap = 16.0 * np.array([math.comb(15, m) for m in range(16)], dtype=np.float64)
    W = []
    for i in range(DIM):
        c = np.cumsum(_softplus64(As[i].astype(np.float64)), axis=1)
        ca = 2.0 * (1.0 / (1.0 + np.exp(-c)) - 0.5)
        ca_ext = np.concatenate(
            [np.zeros((ca.shape[0], 1)), ca, np.ones((ca.shape[0], 1))], axis=1)
        W.append(kap[None, :] * (ca_ext[:, 1:] - ca_ext[:, :-1]))  # [rows,16]

    wcomb = np.zeros((P, 2, 80), dtype=np.float64)
    for h in range(2):
        for p in range(P):
            c4 = 128 * h + p
            wcomb[p, h, 0:16] = W[0][0]
            wcomb[p, h, 16:32] = W[1][c4 >> 6]
            wcomb[p, h, 32:48] = W[2][c4 >> 4]
            wcomb[p, h, 48:64] = W[3][c4 >> 2]
            wcomb[p, h, 64:80] = W[4][c4]
    ca5w = np.zeros((P, 2, 64), dtype=np.float64)
    for h in range(2):
        for p in range(P):
            for l in range(4):
                ca5w[p, h, l * 16:(l + 1) * 16] = W[5][(128 * h + p) * 4 + l]
    return wcomb.astype(np.float16), ca5w.astype(np.float16)


def kernel(**inputs):
    x = np.asarray(inputs["x"], dtype=np.float32)
    As = [np.asarray(inputs[f"A{i}"], dtype=np.float32) for i in range(DIM)]

    if "nc" not in _CACHE:
        _CACHE["nc"] = _build_nc()
    nc = _CACHE["nc"]

    wcomb, ca5w = _host_consts(As)
    idh = np.eye(P, dtype=np.float16)

    in_maps = []
    for c in range(NCORES):
        xc = x[c * NC:(c + 1) * NC].reshape(P, S, DIM)
        in_maps.append({"xr": xc, "wcomb": wcomb, "ca5w": ca5w, "idh": idh})

    res = run_bass_kernel_spmd(nc, in_maps, core_ids=list(range(NCORES)))
    outs = [r["dens"].reshape(NC) for r in res.results]
    return np.concatenate(outs, axis=0)


if __name__ == "__main__":
    rng = np.random.default_rng(0)
    ins = {"x": rng.uniform(0, 1, (N, DIM)).astype(np.float32)}
    for i in range(DIM):
        ins[f"A{i}"] = rng.uniform(0, 1, ((4 ** i), 15)).astype(np.float32)
    out = kernel(**ins)
    print(out.shape, out[:4])
